# revision 3
# baseline (speedup 1.0000x reference)
"""Trainium2 Bass kernel for quantized 1x1-conv + BatchNorm(train) + MultiStepLIF.

Strategy (8 NeuronCores, data-parallel over batch B=16 -> 2 per core):
  y[t,b,o,hw] = sum_c x[t,b,c,hw] * w_int[o,c] * scale_q      (PE, bf16 hi/lo split)
  BN stats (mean/var per o over all t,b,hw)                   (free accum on ACT evict +
                                                               one DVE pass for sum-sq,
                                                               AllReduce of 2x256 floats)
  z = y*inv + (beta - mean*inv);  LIF: v=(v_prev+z)/2, s=[v>=1], reset
  Output per element: m0 = [v < 1] in bf16 (exact complement of spike).

Precision: x is split hi/lo into two bf16 streams (w_int is exact in bf16),
accumulated in fp32 PSUM -> ~1e-5 relative error on y, i.e. fp32-grade.
All LIF/BN elementwise work is fp32.
"""

import os
import sys

for _p in ("/opt/trn_rl_repo",):
    if _p not in sys.path:
        sys.path.insert(0, _p)

import numpy as np
import ml_dtypes

import concourse.bass as bass
import concourse.mybir as mybir
import concourse.tile as tile
from concourse import bacc
from concourse import bass_utils

BF16 = ml_dtypes.bfloat16
F32 = mybir.dt.float32
BF = mybir.dt.bfloat16
ALU = mybir.AluOpType
ACTF = mybir.ActivationFunctionType

T, B, C, H, W = 4, 16, 256, 32, 32
O = 256
NCORES = 8
BC = B // NCORES          # batches per core
TBC = T * BC              # 8 (t,b) pairs per core
HWP = H * W               # 1024
NTOT = float(T * B * H * W)  # positions per channel, global
EPS = 1e-5


def _build_nc():
    nc = bacc.Bacc(
        "TRN2",
        target_bir_lowering=False,
        debug=False,
        num_devices=NCORES,
    )
    xin = nc.dram_tensor("xin", [TBC, 128, 2, 2, HWP], BF, kind="ExternalInput")
    wT = nc.dram_tensor("wT", [128, 2, O], BF, kind="ExternalInput")
    gb = nc.dram_tensor("gb", [128, 6], F32, kind="ExternalInput")
    out = nc.dram_tensor("m0", [T, 2, 128, BC * HWP], BF, kind="ExternalOutput")

    xin_ap = xin.ap()
    out_ap = out.ap()

    with tile.TileContext(nc) as tc:
        with (
            tc.tile_pool(name="consts", bufs=1) as consts,
            tc.tile_pool(name="xpool", bufs=3) as xpool,
            tc.tile_pool(name="ypool", bufs=1) as ypool,
            tc.tile_pool(name="pspool", bufs=4, space="PSUM") as pspool,
            tc.tile_pool(name="scrpool", bufs=2) as scrpool,
            tc.tile_pool(name="small", bufs=1) as small,
            tc.tile_pool(name="dram", bufs=1, space="DRAM") as dram,
            tc.tile_pool(name="upool", bufs=2) as upool,
            tc.tile_pool(name="mpool", bufs=3) as mpool,
            tc.tile_pool(name="cpool", bufs=4) as cpool,
        ):
            # ---- constants ----
            w_sb = consts.tile([128, 2, O], BF, name="w_sb")
            nc.sync.dma_start(w_sb[:], wT.ap())
            gb_sb = consts.tile([128, 6], F32, name="gb_sb")
            nc.sync.dma_start(gb_sb[:], gb.ap())

            # Preload the sqrt ACT table early so the post-collective sqrt
            # doesn't pay the table-switch latency on the critical path.
            junk = small.tile([128, 1], F32, name="junk")
            nc.scalar.activation(junk[:], gb_sb[:, 0:1], ACTF.Sqrt)

            # ---- phase A: matmul + stats ----
            y_sb = [
                ypool.tile([128, TBC * HWP], F32, name=f"ysb{ot}") for ot in range(2)
            ]
            # per-(ot,tb) accumulator columns
            sums = small.tile([128, 2 * TBC], F32, name="sums")
            ssqs = small.tile([128, 2 * TBC], F32, name="ssqs")

            for tb in range(TBC):
                x_t = xpool.tile([128, 2, 2, HWP], BF, name="x_t", tag="x_t")
                nc.sync.dma_start(x_t[:], xin_ap[tb])
                for ot in range(2):
                    ps = pspool.tile([128, HWP], F32, name="ps", tag="ps")
                    for chunk in range(2):
                        nacc = 0
                        for ch in range(2):
                            for part in range(2):
                                nc.tensor.matmul(
                                    ps[:, chunk * 512 : (chunk + 1) * 512],
                                    lhsT=w_sb[:, ch, ot * 128 : (ot + 1) * 128],
                                    rhs=x_t[:, part, ch, chunk * 512 : (chunk + 1) * 512],
                                    start=(nacc == 0),
                                    stop=(nacc == 3),
                                )
                                nacc += 1
                    ysl = y_sb[ot][:, tb * HWP : (tb + 1) * HWP]
                    col = 2 * tb + ot
                    # evict PSUM -> SBUF in real units (scale_q) + free row-sum
                    nc.scalar.activation(
                        ysl,
                        ps[:],
                        ACTF.Copy,
                        bias=0.0,
                        scale=gb_sb[:, 4:5],
                        accum_out=sums[:, col : col + 1],
                    )
                    # sum of squares in one DVE pass (output discarded)
                    scr = scrpool.tile([128, HWP], F32, name="scr", tag="scr")
                    nc.vector.scalar_tensor_tensor(
                        out=scr[:],
                        in0=ysl,
                        scalar=0.0,
                        in1=ysl,
                        op0=ALU.bypass,
                        op1=ALU.mult,
                        accum_out=ssqs[:, col : col + 1],
                    )

            # ---- finalize local stats, AllReduce ----
            stats4 = small.tile([128, 4], F32, name="stats4")
            nc.vector.tensor_reduce(
                stats4[:, 0:2],
                sums[:].rearrange("p (t o) -> p o t", o=2),
                axis=mybir.AxisListType.X,
                op=ALU.add,
            )
            nc.vector.tensor_reduce(
                stats4[:, 2:4],
                ssqs[:].rearrange("p (t o) -> p o t", o=2),
                axis=mybir.AxisListType.X,
                op=ALU.add,
            )

            cc_in = dram.tile([128, 4], F32, name="cc_in")
            cc_out = dram.tile([128, 4], F32, name="cc_out")
            nc.gpsimd.dma_start(cc_in[:], stats4[:])
            nc.gpsimd.collective_compute(
                "AllReduce",
                ALU.add,
                replica_groups=[list(range(NCORES))],
                ins=[cc_in.opt()],
                outs=[cc_out.opt()],
            )
            gstat = small.tile([128, 4], F32, name="gstat")
            nc.gpsimd.dma_start(gstat[:], cc_out[:])

            # ---- small math: a = 0.5*inv, b = 0.5*(beta - mean*inv) ----
            aab = small.tile([128, 4], F32, name="aab")
            mean = small.tile([128, 2], F32, name="mean")
            e2 = small.tile([128, 2], F32, name="e2")
            msq = small.tile([128, 2], F32, name="msq")
            vare = small.tile([128, 2], F32, name="vare")
            sq = small.tile([128, 2], F32, name="sq")
            dd = small.tile([128, 2], F32, name="dd")
            t1 = small.tile([128, 2], F32, name="t1")
            inv = small.tile([128, 2], F32, name="inv")
            mi = small.tile([128, 2], F32, name="mi")
            bh = small.tile([128, 2], F32, name="bh")

            inv_n = 1.0 / NTOT
            nc.vector.tensor_scalar(mean[:], gstat[:, 0:2], inv_n, None, ALU.mult)
            nc.vector.tensor_scalar(e2[:], gstat[:, 2:4], inv_n, None, ALU.mult)
            nc.vector.tensor_tensor(msq[:], mean[:], mean[:], ALU.mult)
            # vare = e2 - mean^2 + eps
            nc.vector.scalar_tensor_tensor(
                out=t1[:], in0=msq[:], scalar=-1.0, in1=e2[:], op0=ALU.mult, op1=ALU.add
            )
            nc.vector.tensor_scalar(vare[:], t1[:], EPS, None, ALU.add)
            nc.scalar.activation(sq[:], vare[:], ACTF.Sqrt)
            # two Newton refinements: s = 0.5*(s + vare/s)
            rs = small.tile([128, 2], F32, name="rs")
            for _ in range(2):
                nc.vector.reciprocal(rs[:], sq[:])
                nc.vector.tensor_tensor(dd[:], vare[:], rs[:], ALU.mult)
                nc.vector.tensor_tensor(t1[:], sq[:], dd[:], ALU.add)
                nc.vector.tensor_scalar(sq[:], t1[:], 0.5, None, ALU.mult)
            # inv = gamma / sqrt(var+eps)
            nc.vector.reciprocal(rs[:], sq[:])
            nc.vector.tensor_tensor(inv[:], gb_sb[:, 0:2], rs[:], ALU.mult)
            nc.vector.tensor_scalar(aab[:, 0:2], inv[:], 0.5, None, ALU.mult)
            nc.vector.tensor_tensor(mi[:], mean[:], inv[:], ALU.mult)
            nc.vector.tensor_scalar(bh[:], gb_sb[:, 2:4], 0.5, None, ALU.mult)
            nc.vector.scalar_tensor_tensor(
                out=aab[:, 2:4], in0=mi[:], scalar=-0.5, in1=bh[:], op0=ALU.mult, op1=ALU.add
            )

            # ---- phase B: LIF over T ----
            carry = [None, None]
            for t in range(T):
                for ot in range(2):
                    ysl = y_sb[ot][:, t * BC * HWP : (t + 1) * BC * HWP]
                    u = upool.tile([128, BC * HWP], F32, name="u", tag="u")
                    nc.scalar.activation(
                        u[:],
                        ysl,
                        ACTF.Identity,
                        bias=aab[:, 2 + ot : 3 + ot],
                        scale=aab[:, ot : ot + 1],
                    )
                    if t > 0:
                        # v = u + carry  (in place)
                        nc.vector.tensor_tensor(u[:], u[:], carry[ot][:], ALU.add)
                    m0 = mpool.tile([128, BC * HWP], BF, name="m0", tag="m0")
                    nc.gpsimd.tensor_scalar(m0[:], u[:], 1.0, None, ALU.is_lt)
                    nc.sync.dma_start(out_ap[t, ot], m0[:])
                    if t < T - 1:
                        cnew = cpool.tile([128, BC * HWP], F32, name="carry", tag="carry")
                        nc.vector.scalar_tensor_tensor(
                            out=cnew[:],
                            in0=u[:],
                            scalar=0.5,
                            in1=m0[:],
                            op0=ALU.mult,
                            op1=ALU.mult,
                        )
                        carry[ot] = cnew

    nc.compile()
    return nc


_NC_CACHE = None


def _get_nc():
    global _NC_CACHE
    if _NC_CACHE is None:
        _NC_CACHE = _build_nc()
    return _NC_CACHE


def _prep_inputs(x, w, gamma, beta):
    x = np.ascontiguousarray(np.asarray(x, dtype=np.float32))
    w = np.asarray(w, dtype=np.float32)
    gamma = np.asarray(gamma, dtype=np.float32)
    beta = np.asarray(beta, dtype=np.float32)

    # fake-quant weights exactly like the reference forward pass
    scale = (np.max(np.abs(w)) / np.float32(127.0)).astype(np.float32)
    wint = np.clip(np.rint((w / scale).astype(np.float32)), -127.0, 127.0).astype(
        np.float32
    )
    # lhsT layout: [cc(128), ch(2), O]  (w_int values are exact in bf16)
    wT_packed = np.ascontiguousarray(
        wint.T.reshape(2, 128, O).transpose(1, 0, 2)
    ).astype(BF16)

    gb_packed = np.zeros((128, 6), np.float32)
    gb_packed[:, 0] = gamma[:128]
    gb_packed[:, 1] = gamma[128:]
    gb_packed[:, 2] = beta[:128]
    gb_packed[:, 3] = beta[128:]
    gb_packed[:, 4] = scale

    xs = x.reshape(T, NCORES, BC, C, HWP)
    in_maps = []
    for c in range(NCORES):
        xf = np.ascontiguousarray(xs[:, c]).reshape(T * BC, 2, 128, HWP)
        hi = xf.astype(BF16)
        lo = (xf - hi.astype(np.float32)).astype(BF16)
        xin = np.stack([hi, lo], axis=1)  # [tb, part, ch, cc, hw]
        xin = np.ascontiguousarray(xin.transpose(0, 3, 1, 2, 4))  # [tb, cc, part, ch, hw]
        in_maps.append({"xin": xin, "wT": wT_packed, "gb": gb_packed})
    return in_maps


def _assemble(results):
    spikes = np.empty((T, B, O, H, W), np.float32)
    for c in range(NCORES):
        m = results[c]["m0"].astype(np.float32)  # [T, 2, 128, BC*HWP]
        s = 1.0 - m
        sm = s.reshape(T, 2, 128, BC, HWP).transpose(0, 3, 1, 2, 4)  # [t,b,ot,oc,hw]
        spikes[:, c * BC : (c + 1) * BC] = sm.reshape(T, BC, O, H, W)
    return spikes


def run(x, w, gamma, beta, trace=False, **spmd_kwargs):
    in_maps = _prep_inputs(x, w, gamma, beta)
    nc = _get_nc()
    res = bass_utils.run_bass_kernel_spmd(
        nc, in_maps, core_ids=list(range(NCORES)), trace=trace, **spmd_kwargs
    )
    return _assemble(res.results), res


def kernel(x, w, gamma, beta):
    spikes, _ = run(x, w, gamma, beta)
    return spikes


# revision 5
# speedup vs baseline: 2.7091x; 2.7091x over previous
"""Trainium2 Bass kernel for quantized 1x1-conv + BatchNorm(train) + MultiStepLIF.

Strategy (8 NeuronCores, data-parallel over batch B=16 -> 2 per core):
  y[t,b,o,hw] = sum_c x[t,b,c,hw] * w_int[o,c] * scale_q      (PE, bf16 hi/lo split)
  BN stats (mean/var per o over all t,b,hw)                   (free accum on ACT evict +
                                                               one DVE pass for sum-sq,
                                                               AllReduce of 2x256 floats)
  z = y*inv + (beta - mean*inv);  LIF: v=(v_prev+z)/2, s=[v>=1], reset
  Output per element: m0 = [v < 1] in bf16 (exact complement of spike).

Precision: x is split hi/lo into two bf16 streams (w_int is exact in bf16),
accumulated in fp32 PSUM -> ~1e-5 relative error on y, i.e. fp32-grade.
All LIF/BN elementwise work is fp32.
"""

import os
import sys

for _p in ("/opt/trn_rl_repo",):
    if _p not in sys.path:
        sys.path.insert(0, _p)

import numpy as np
import ml_dtypes

import concourse.bass as bass
import concourse.mybir as mybir
import concourse.tile as tile
from concourse import bacc
from concourse import bass_utils

BF16 = ml_dtypes.bfloat16
F32 = mybir.dt.float32
BF = mybir.dt.bfloat16
ALU = mybir.AluOpType
ACTF = mybir.ActivationFunctionType

T, B, C, H, W = 4, 16, 256, 32, 32
O = 256
NCORES = 8
BC = B // NCORES          # batches per core
TBC = T * BC              # 8 (t,b) pairs per core
HWP = H * W               # 1024
NTOT = float(T * B * H * W)  # positions per channel, global
EPS = 1e-5


def _build_nc():
    nc = bacc.Bacc(
        "TRN2",
        target_bir_lowering=False,
        debug=False,
        num_devices=NCORES,
    )
    xin = nc.dram_tensor("xin", [TBC, 128, 2, 2, HWP], BF, kind="ExternalInput")
    wT = nc.dram_tensor("wT", [128, 2, O], BF, kind="ExternalInput")
    gb = nc.dram_tensor("gb", [128, 6], F32, kind="ExternalInput")
    out = nc.dram_tensor("m0", [T, 2, 128, BC * HWP], BF, kind="ExternalOutput")

    xin_ap = xin.ap()
    out_ap = out.ap()

    with tile.TileContext(nc) as tc:
        with (
            tc.tile_pool(name="consts", bufs=1) as consts,
            tc.tile_pool(name="xpool", bufs=3) as xpool,
            tc.tile_pool(name="ypool", bufs=1) as ypool,
            tc.tile_pool(name="pspool", bufs=4, space="PSUM") as pspool,
            tc.tile_pool(name="scrpool", bufs=2) as scrpool,
            tc.tile_pool(name="small", bufs=1) as small,
            tc.tile_pool(name="dram", bufs=1, space="DRAM") as dram,
            tc.tile_pool(name="upool", bufs=2) as upool,
            tc.tile_pool(name="mpool", bufs=3) as mpool,
            tc.tile_pool(name="cpool", bufs=4) as cpool,
        ):
            # ---- constants ----
            w_sb = consts.tile([128, 2, O], BF, name="w_sb")
            nc.sync.dma_start(w_sb[:], wT.ap())
            gb_sb = consts.tile([128, 6], F32, name="gb_sb")
            nc.sync.dma_start(gb_sb[:], gb.ap())

            # Preload the sqrt ACT table early so the post-collective sqrt
            # doesn't pay the table-switch latency on the critical path.
            junk = small.tile([128, 1], F32, name="junk")
            nc.scalar.activation(junk[:], gb_sb[:, 0:1], ACTF.Sqrt)

            # ---- phase A: matmul + stats ----
            y_sb = [
                ypool.tile([128, TBC * HWP], F32, name=f"ysb{ot}") for ot in range(2)
            ]
            # per-(ot,tb) accumulator columns
            sums = small.tile([128, 2 * TBC], F32, name="sums")
            ssqs = small.tile([128, 2 * TBC], F32, name="ssqs")

            for tb in range(TBC):
                x_t = xpool.tile([128, 2, 2, HWP], BF, name="x_t", tag="x_t")
                nc.sync.dma_start(x_t[:], xin_ap[tb])
                for ot in range(2):
                    ps = pspool.tile([128, HWP], F32, name="ps", tag="ps")
                    for chunk in range(2):
                        nacc = 0
                        for ch in range(2):
                            for part in range(2):
                                nc.tensor.matmul(
                                    ps[:, chunk * 512 : (chunk + 1) * 512],
                                    lhsT=w_sb[:, ch, ot * 128 : (ot + 1) * 128],
                                    rhs=x_t[:, part, ch, chunk * 512 : (chunk + 1) * 512],
                                    start=(nacc == 0),
                                    stop=(nacc == 3),
                                )
                                nacc += 1
                    ysl = y_sb[ot][:, tb * HWP : (tb + 1) * HWP]
                    col = 2 * tb + ot
                    # evict PSUM -> SBUF in real units (scale_q) + free row-sum
                    nc.scalar.activation(
                        ysl,
                        ps[:],
                        ACTF.Copy,
                        bias=0.0,
                        scale=gb_sb[:, 4:5],
                        accum_out=sums[:, col : col + 1],
                    )
                    # sum of squares in one DVE pass (output discarded)
                    scr = scrpool.tile([128, HWP], F32, name="scr", tag="scr")
                    nc.vector.scalar_tensor_tensor(
                        out=scr[:],
                        in0=ysl,
                        scalar=0.0,
                        in1=ysl,
                        op0=ALU.bypass,
                        op1=ALU.mult,
                        accum_out=ssqs[:, col : col + 1],
                    )

            # ---- finalize local stats, AllReduce ----
            stats4 = small.tile([128, 4], F32, name="stats4")
            nc.vector.tensor_reduce(
                stats4[:, 0:2],
                sums[:].rearrange("p (t o) -> p o t", o=2),
                axis=mybir.AxisListType.X,
                op=ALU.add,
            )
            nc.vector.tensor_reduce(
                stats4[:, 2:4],
                ssqs[:].rearrange("p (t o) -> p o t", o=2),
                axis=mybir.AxisListType.X,
                op=ALU.add,
            )

            cc_in = dram.tile([128, 4], F32, name="cc_in")
            cc_out = dram.tile([128, 4], F32, name="cc_out")
            nc.gpsimd.dma_start(cc_in[:], stats4[:])
            nc.gpsimd.collective_compute(
                "AllReduce",
                ALU.add,
                replica_groups=[list(range(NCORES))],
                ins=[cc_in.opt()],
                outs=[cc_out.opt()],
            )
            gstat = small.tile([128, 4], F32, name="gstat")
            nc.gpsimd.dma_start(gstat[:], cc_out[:])

            # ---- small math: a = 0.5*inv, b = 0.5*(beta - mean*inv) ----
            aab = small.tile([128, 4], F32, name="aab")
            mean = small.tile([128, 2], F32, name="mean")
            e2 = small.tile([128, 2], F32, name="e2")
            msq = small.tile([128, 2], F32, name="msq")
            vare = small.tile([128, 2], F32, name="vare")
            sq = small.tile([128, 2], F32, name="sq")
            dd = small.tile([128, 2], F32, name="dd")
            t1 = small.tile([128, 2], F32, name="t1")
            inv = small.tile([128, 2], F32, name="inv")
            mi = small.tile([128, 2], F32, name="mi")
            bh = small.tile([128, 2], F32, name="bh")

            inv_n = 1.0 / NTOT
            nc.vector.tensor_scalar(mean[:], gstat[:, 0:2], inv_n, None, ALU.mult)
            nc.vector.tensor_scalar(e2[:], gstat[:, 2:4], inv_n, None, ALU.mult)
            nc.vector.tensor_tensor(msq[:], mean[:], mean[:], ALU.mult)
            # vare = e2 - mean^2 + eps
            nc.vector.scalar_tensor_tensor(
                out=t1[:], in0=msq[:], scalar=-1.0, in1=e2[:], op0=ALU.mult, op1=ALU.add
            )
            nc.vector.tensor_scalar(vare[:], t1[:], EPS, None, ALU.add)
            nc.scalar.activation(sq[:], vare[:], ACTF.Sqrt)
            # two Newton refinements: s = 0.5*(s + vare/s)
            rs = small.tile([128, 2], F32, name="rs")
            for _ in range(2):
                nc.vector.reciprocal(rs[:], sq[:])
                nc.vector.tensor_tensor(dd[:], vare[:], rs[:], ALU.mult)
                nc.vector.tensor_tensor(t1[:], sq[:], dd[:], ALU.add)
                nc.vector.tensor_scalar(sq[:], t1[:], 0.5, None, ALU.mult)
            # inv = gamma / sqrt(var+eps)
            nc.vector.reciprocal(rs[:], sq[:])
            nc.vector.tensor_tensor(inv[:], gb_sb[:, 0:2], rs[:], ALU.mult)
            nc.vector.tensor_scalar(aab[:, 0:2], inv[:], 0.5, None, ALU.mult)
            nc.vector.tensor_tensor(mi[:], mean[:], inv[:], ALU.mult)
            nc.vector.tensor_scalar(bh[:], gb_sb[:, 2:4], 0.5, None, ALU.mult)
            nc.vector.scalar_tensor_tensor(
                out=aab[:, 2:4], in0=mi[:], scalar=-0.5, in1=bh[:], op0=ALU.mult, op1=ALU.add
            )

            # ---- phase B: LIF over T ----
            carry = [None, None]
            for t in range(T):
                for ot in range(2):
                    ysl = y_sb[ot][:, t * BC * HWP : (t + 1) * BC * HWP]
                    u = upool.tile([128, BC * HWP], F32, name="u", tag="u")
                    nc.scalar.activation(
                        u[:],
                        ysl,
                        ACTF.Identity,
                        bias=aab[:, 2 + ot : 3 + ot],
                        scale=aab[:, ot : ot + 1],
                    )
                    if t > 0:
                        # v = u + carry  (in place)
                        nc.vector.tensor_tensor(u[:], u[:], carry[ot][:], ALU.add)
                    # m = 0.5 where v < 1 else 0  (bf16-exact; host: spike = m==0)
                    m0 = mpool.tile([128, BC * HWP], BF, name="m0", tag="m0")
                    nc.vector.tensor_scalar(
                        m0[:], u[:], 1.0, 0.5, ALU.is_lt, ALU.mult
                    )
                    nc.sync.dma_start(out_ap[t, ot], m0[:])
                    if t < T - 1:
                        cnew = cpool.tile([128, BC * HWP], F32, name="carry", tag="carry")
                        nc.vector.tensor_tensor(cnew[:], u[:], m0[:], ALU.mult)
                        carry[ot] = cnew

    nc.compile()
    return nc


_NC_CACHE = None


def _get_nc():
    global _NC_CACHE
    if _NC_CACHE is None:
        _NC_CACHE = _build_nc()
    return _NC_CACHE


def _prep_inputs(x, w, gamma, beta):
    x = np.ascontiguousarray(np.asarray(x, dtype=np.float32))
    w = np.asarray(w, dtype=np.float32)
    gamma = np.asarray(gamma, dtype=np.float32)
    beta = np.asarray(beta, dtype=np.float32)

    # fake-quant weights exactly like the reference forward pass
    scale = (np.max(np.abs(w)) / np.float32(127.0)).astype(np.float32)
    wint = np.clip(np.rint((w / scale).astype(np.float32)), -127.0, 127.0).astype(
        np.float32
    )
    # lhsT layout: [cc(128), ch(2), O]  (w_int values are exact in bf16)
    wT_packed = np.ascontiguousarray(
        wint.T.reshape(2, 128, O).transpose(1, 0, 2)
    ).astype(BF16)

    gb_packed = np.zeros((128, 6), np.float32)
    gb_packed[:, 0] = gamma[:128]
    gb_packed[:, 1] = gamma[128:]
    gb_packed[:, 2] = beta[:128]
    gb_packed[:, 3] = beta[128:]
    gb_packed[:, 4] = scale

    xs = x.reshape(T, NCORES, BC, C, HWP)
    in_maps = []
    for c in range(NCORES):
        xf = np.ascontiguousarray(xs[:, c]).reshape(T * BC, 2, 128, HWP)
        hi = xf.astype(BF16)
        lo = (xf - hi.astype(np.float32)).astype(BF16)
        xin = np.stack([hi, lo], axis=1)  # [tb, part, ch, cc, hw]
        xin = np.ascontiguousarray(xin.transpose(0, 3, 1, 2, 4))  # [tb, cc, part, ch, hw]
        in_maps.append({"xin": xin, "wT": wT_packed, "gb": gb_packed})
    return in_maps


def _assemble(results):
    spikes = np.empty((T, B, O, H, W), np.float32)
    for c in range(NCORES):
        m = results[c]["m0"]  # [T, 2, 128, BC*HWP] bf16, values {0, 0.5}
        s = (m == 0).astype(np.float32)
        sm = s.reshape(T, 2, 128, BC, HWP).transpose(0, 3, 1, 2, 4)  # [t,b,ot,oc,hw]
        spikes[:, c * BC : (c + 1) * BC] = sm.reshape(T, BC, O, H, W)
    return spikes


def run(x, w, gamma, beta, trace=False, **spmd_kwargs):
    in_maps = _prep_inputs(x, w, gamma, beta)
    nc = _get_nc()
    res = bass_utils.run_bass_kernel_spmd(
        nc, in_maps, core_ids=list(range(NCORES)), trace=trace, **spmd_kwargs
    )
    return _assemble(res.results), res


def kernel(x, w, gamma, beta):
    spikes, _ = run(x, w, gamma, beta)
    return spikes


# revision 6
# speedup vs baseline: 2.7545x; 1.0168x over previous
"""Trainium2 Bass kernel for quantized 1x1-conv + BatchNorm(train) + MultiStepLIF.

Strategy (8 NeuronCores, data-parallel over batch B=16 -> 2 per core):
  y[t,b,o,hw] = sum_c x[t,b,c,hw] * w_int[o,c] * scale_q      (PE, bf16 hi/lo split)
  BN stats (mean/var per o over all t,b,hw)                   (free accum on ACT evict +
                                                               one DVE pass for sum-sq,
                                                               AllReduce of 2x256 floats)
  z = y*inv + (beta - mean*inv);  LIF: v=(v_prev+z)/2, s=[v>=1], reset
  Output per element: m0 = [v < 1] in bf16 (exact complement of spike).

Precision: x is split hi/lo into two bf16 streams (w_int is exact in bf16),
accumulated in fp32 PSUM -> ~1e-5 relative error on y, i.e. fp32-grade.
All LIF/BN elementwise work is fp32.
"""

import os
import sys

for _p in ("/opt/trn_rl_repo",):
    if _p not in sys.path:
        sys.path.insert(0, _p)

import numpy as np
import ml_dtypes

import concourse.bass as bass
import concourse.mybir as mybir
import concourse.tile as tile
from concourse import bacc
from concourse import bass_utils

BF16 = ml_dtypes.bfloat16
F32 = mybir.dt.float32
BF = mybir.dt.bfloat16
ALU = mybir.AluOpType
ACTF = mybir.ActivationFunctionType

T, B, C, H, W = 4, 16, 256, 32, 32
O = 256
NCORES = 8
BC = B // NCORES          # batches per core
TBC = T * BC              # 8 (t,b) pairs per core
HWP = H * W               # 1024
NTOT = float(T * B * H * W)  # positions per channel, global
EPS = 1e-5


def _build_nc():
    nc = bacc.Bacc(
        "TRN2",
        target_bir_lowering=False,
        debug=False,
        num_devices=NCORES,
    )
    xin = nc.dram_tensor("xin", [TBC, 128, 2, 2, HWP], BF, kind="ExternalInput")
    wT = nc.dram_tensor("wT", [128, 2, O], BF, kind="ExternalInput")
    gb = nc.dram_tensor("gb", [128, 6], F32, kind="ExternalInput")
    out = nc.dram_tensor("m0", [T, 2, 128, BC * HWP], BF, kind="ExternalOutput")

    xin_ap = xin.ap()
    out_ap = out.ap()

    with tile.TileContext(nc) as tc:
        with (
            tc.tile_pool(name="consts", bufs=1) as consts,
            tc.tile_pool(name="xpool", bufs=3) as xpool,
            tc.tile_pool(name="ypool", bufs=1) as ypool,
            tc.tile_pool(name="pspool", bufs=4, space="PSUM") as pspool,
            tc.tile_pool(name="scrpool", bufs=2) as scrpool,
            tc.tile_pool(name="small", bufs=1) as small,
            tc.tile_pool(name="dram", bufs=1, space="DRAM") as dram,
            tc.tile_pool(name="upool", bufs=2) as upool,
            tc.tile_pool(name="mpool", bufs=3) as mpool,
            tc.tile_pool(name="cpool", bufs=4) as cpool,
        ):
            # ---- constants ----
            w_sb = consts.tile([128, 2, O], BF, name="w_sb")
            nc.sync.dma_start(w_sb[:], wT.ap())
            gb_sb = consts.tile([128, 6], F32, name="gb_sb")
            nc.sync.dma_start(gb_sb[:], gb.ap())

            # Preload the sqrt ACT table early so the post-collective sqrt
            # doesn't pay the table-switch latency on the critical path.
            junk = small.tile([128, 1], F32, name="junk")
            nc.scalar.activation(junk[:], gb_sb[:, 0:1], ACTF.Sqrt)

            # ---- phase A: matmul + stats ----
            y_sb = [
                ypool.tile([128, TBC * HWP], F32, name=f"ysb{ot}") for ot in range(2)
            ]
            # per-(ot,tb) accumulator columns
            sums = small.tile([128, 2 * TBC], F32, name="sums")
            ssqs = small.tile([128, 2 * TBC], F32, name="ssqs")

            for tb in range(TBC):
                x_t = xpool.tile([128, 2, 2, HWP], BF, name="x_t", tag="x_t")
                nc.sync.dma_start(x_t[:], xin_ap[tb])
                for ot in range(2):
                    ps = pspool.tile([128, HWP], F32, name="ps", tag="ps")
                    for chunk in range(2):
                        nacc = 0
                        for ch in range(2):
                            for part in range(2):
                                nc.tensor.matmul(
                                    ps[:, chunk * 512 : (chunk + 1) * 512],
                                    lhsT=w_sb[:, ch, ot * 128 : (ot + 1) * 128],
                                    rhs=x_t[:, part, ch, chunk * 512 : (chunk + 1) * 512],
                                    start=(nacc == 0),
                                    stop=(nacc == 3),
                                )
                                nacc += 1
                    ysl = y_sb[ot][:, tb * HWP : (tb + 1) * HWP]
                    col = 2 * tb + ot
                    # evict PSUM -> SBUF in real units (scale_q) + free row-sum
                    nc.scalar.activation(
                        ysl,
                        ps[:],
                        ACTF.Copy,
                        bias=0.0,
                        scale=gb_sb[:, 4:5],
                        accum_out=sums[:, col : col + 1],
                    )
                    # sum of squares in one DVE pass (output discarded)
                    scr = scrpool.tile([128, HWP], F32, name="scr", tag="scr")
                    nc.vector.scalar_tensor_tensor(
                        out=scr[:],
                        in0=ysl,
                        scalar=0.0,
                        in1=ysl,
                        op0=ALU.bypass,
                        op1=ALU.mult,
                        accum_out=ssqs[:, col : col + 1],
                    )

            # ---- finalize local stats, AllReduce ----
            stats4 = small.tile([128, 4], F32, name="stats4")
            nc.vector.tensor_reduce(
                stats4[:, 0:2],
                sums[:].rearrange("p (t o) -> p o t", o=2),
                axis=mybir.AxisListType.X,
                op=ALU.add,
            )
            nc.vector.tensor_reduce(
                stats4[:, 2:4],
                ssqs[:].rearrange("p (t o) -> p o t", o=2),
                axis=mybir.AxisListType.X,
                op=ALU.add,
            )

            cc_in = dram.tile([128, 4], F32, name="cc_in")
            cc_out = dram.tile([NCORES, 128, 4], F32, name="cc_out")
            nc.gpsimd.dma_start(cc_in[:], stats4[:])
            nc.gpsimd.collective_compute(
                "AllGather",
                ALU.bypass,
                replica_groups=[list(range(NCORES))],
                ins=[cc_in.opt()],
                outs=[cc_out.opt()],
            )
            g8 = small.tile([128, NCORES, 4], F32, name="g8")
            nc.gpsimd.dma_start(g8[:], cc_out[:].rearrange("r p c -> p r c"))
            gstat = small.tile([128, 4], F32, name="gstat")
            nc.vector.tensor_reduce(
                gstat[:],
                g8[:].rearrange("p r c -> p c r"),
                axis=mybir.AxisListType.X,
                op=ALU.add,
            )

            # ---- small math: a = 0.5*inv, b = 0.5*(beta - mean*inv) ----
            aab = small.tile([128, 4], F32, name="aab")
            mean = small.tile([128, 2], F32, name="mean")
            e2 = small.tile([128, 2], F32, name="e2")
            msq = small.tile([128, 2], F32, name="msq")
            vare = small.tile([128, 2], F32, name="vare")
            sq = small.tile([128, 2], F32, name="sq")
            dd = small.tile([128, 2], F32, name="dd")
            t1 = small.tile([128, 2], F32, name="t1")
            inv = small.tile([128, 2], F32, name="inv")
            mi = small.tile([128, 2], F32, name="mi")
            bh = small.tile([128, 2], F32, name="bh")

            inv_n = 1.0 / NTOT
            nc.vector.tensor_scalar(mean[:], gstat[:, 0:2], inv_n, None, ALU.mult)
            nc.vector.tensor_scalar(e2[:], gstat[:, 2:4], inv_n, None, ALU.mult)
            nc.vector.tensor_tensor(msq[:], mean[:], mean[:], ALU.mult)
            # vare = e2 - mean^2 + eps
            nc.vector.scalar_tensor_tensor(
                out=t1[:], in0=msq[:], scalar=-1.0, in1=e2[:], op0=ALU.mult, op1=ALU.add
            )
            nc.vector.tensor_scalar(vare[:], t1[:], EPS, None, ALU.add)
            nc.scalar.activation(sq[:], vare[:], ACTF.Sqrt)
            # two Newton refinements: s = 0.5*(s + vare/s)
            rs = small.tile([128, 2], F32, name="rs")
            for _ in range(2):
                nc.vector.reciprocal(rs[:], sq[:])
                nc.vector.tensor_tensor(dd[:], vare[:], rs[:], ALU.mult)
                nc.vector.tensor_tensor(t1[:], sq[:], dd[:], ALU.add)
                nc.vector.tensor_scalar(sq[:], t1[:], 0.5, None, ALU.mult)
            # inv = gamma / sqrt(var+eps)
            nc.vector.reciprocal(rs[:], sq[:])
            nc.vector.tensor_tensor(inv[:], gb_sb[:, 0:2], rs[:], ALU.mult)
            nc.vector.tensor_scalar(aab[:, 0:2], inv[:], 0.5, None, ALU.mult)
            nc.vector.tensor_tensor(mi[:], mean[:], inv[:], ALU.mult)
            nc.vector.tensor_scalar(bh[:], gb_sb[:, 2:4], 0.5, None, ALU.mult)
            nc.vector.scalar_tensor_tensor(
                out=aab[:, 2:4], in0=mi[:], scalar=-0.5, in1=bh[:], op0=ALU.mult, op1=ALU.add
            )

            # ---- phase B: LIF over T ----
            carry = [None, None]
            for t in range(T):
                for ot in range(2):
                    ysl = y_sb[ot][:, t * BC * HWP : (t + 1) * BC * HWP]
                    u = upool.tile([128, BC * HWP], F32, name="u", tag="u")
                    nc.scalar.activation(
                        u[:],
                        ysl,
                        ACTF.Identity,
                        bias=aab[:, 2 + ot : 3 + ot],
                        scale=aab[:, ot : ot + 1],
                    )
                    if t > 0:
                        # v = u + carry  (in place)
                        nc.vector.tensor_tensor(u[:], u[:], carry[ot][:], ALU.add)
                    # m = 0.5 where v < 1 else 0  (bf16-exact; host: spike = m==0)
                    m0 = mpool.tile([128, BC * HWP], BF, name="m0", tag="m0")
                    nc.vector.tensor_scalar(
                        m0[:], u[:], 1.0, 0.5, ALU.is_lt, ALU.mult
                    )
                    nc.sync.dma_start(out_ap[t, ot], m0[:])
                    if t < T - 1:
                        cnew = cpool.tile([128, BC * HWP], F32, name="carry", tag="carry")
                        nc.vector.tensor_tensor(cnew[:], u[:], m0[:], ALU.mult)
                        carry[ot] = cnew

    nc.compile()
    return nc


_NC_CACHE = None


def _get_nc():
    global _NC_CACHE
    if _NC_CACHE is None:
        _NC_CACHE = _build_nc()
    return _NC_CACHE


def _prep_inputs(x, w, gamma, beta):
    x = np.ascontiguousarray(np.asarray(x, dtype=np.float32))
    w = np.asarray(w, dtype=np.float32)
    gamma = np.asarray(gamma, dtype=np.float32)
    beta = np.asarray(beta, dtype=np.float32)

    # fake-quant weights exactly like the reference forward pass
    scale = (np.max(np.abs(w)) / np.float32(127.0)).astype(np.float32)
    wint = np.clip(np.rint((w / scale).astype(np.float32)), -127.0, 127.0).astype(
        np.float32
    )
    # lhsT layout: [cc(128), ch(2), O]  (w_int values are exact in bf16)
    wT_packed = np.ascontiguousarray(
        wint.T.reshape(2, 128, O).transpose(1, 0, 2)
    ).astype(BF16)

    gb_packed = np.zeros((128, 6), np.float32)
    gb_packed[:, 0] = gamma[:128]
    gb_packed[:, 1] = gamma[128:]
    gb_packed[:, 2] = beta[:128]
    gb_packed[:, 3] = beta[128:]
    gb_packed[:, 4] = scale

    xs = x.reshape(T, NCORES, BC, C, HWP)
    in_maps = []
    for c in range(NCORES):
        xf = np.ascontiguousarray(xs[:, c]).reshape(T * BC, 2, 128, HWP)
        hi = xf.astype(BF16)
        lo = (xf - hi.astype(np.float32)).astype(BF16)
        xin = np.stack([hi, lo], axis=1)  # [tb, part, ch, cc, hw]
        xin = np.ascontiguousarray(xin.transpose(0, 3, 1, 2, 4))  # [tb, cc, part, ch, hw]
        in_maps.append({"xin": xin, "wT": wT_packed, "gb": gb_packed})
    return in_maps


def _assemble(results):
    spikes = np.empty((T, B, O, H, W), np.float32)
    for c in range(NCORES):
        m = results[c]["m0"]  # [T, 2, 128, BC*HWP] bf16, values {0, 0.5}
        s = (m == 0).astype(np.float32)
        sm = s.reshape(T, 2, 128, BC, HWP).transpose(0, 3, 1, 2, 4)  # [t,b,ot,oc,hw]
        spikes[:, c * BC : (c + 1) * BC] = sm.reshape(T, BC, O, H, W)
    return spikes


def run(x, w, gamma, beta, trace=False, **spmd_kwargs):
    in_maps = _prep_inputs(x, w, gamma, beta)
    nc = _get_nc()
    res = bass_utils.run_bass_kernel_spmd(
        nc, in_maps, core_ids=list(range(NCORES)), trace=trace, **spmd_kwargs
    )
    return _assemble(res.results), res


def kernel(x, w, gamma, beta):
    spikes, _ = run(x, w, gamma, beta)
    return spikes


# revision 11
# speedup vs baseline: 2.8862x; 1.0478x over previous
"""Trainium2 Bass kernel for quantized 1x1-conv + BatchNorm(train) + MultiStepLIF.

Strategy (8 NeuronCores, data-parallel over batch B=16 -> 2 per core):
  y[t,b,o,hw] = sum_c x[t,b,c,hw] * w_int[o,c] * scale_q      (PE, bf16 hi/lo split)
  BN stats (mean/var per o over all t,b,hw)                   (free accum on ACT evict +
                                                               one DVE pass for sum-sq,
                                                               AllReduce of 2x256 floats)
  z = y*inv + (beta - mean*inv);  LIF: v=(v_prev+z)/2, s=[v>=1], reset
  Output per element: m0 = [v < 1] in bf16 (exact complement of spike).

Precision: x is split hi/lo into two bf16 streams (w_int is exact in bf16),
accumulated in fp32 PSUM -> ~1e-5 relative error on y, i.e. fp32-grade.
All LIF/BN elementwise work is fp32.
"""

import os
import sys

for _p in ("/opt/trn_rl_repo",):
    if _p not in sys.path:
        sys.path.insert(0, _p)

import numpy as np
import ml_dtypes

import concourse.bass as bass
import concourse.mybir as mybir
import concourse.tile as tile
from concourse import bacc
from concourse import bass_utils
from concourse import dve_ops as _dve_ops
from concourse.dve_spec import C0, C1, Spec as _DveSpec, Src0, Src1, Zero, select as _dve_select

BF16 = ml_dtypes.bfloat16
F32 = mybir.dt.float32
BF = mybir.dt.bfloat16
ALU = mybir.AluOpType
ACTF = mybir.ActivationFunctionType

T, B, C, H, W = 4, 16, 256, 32, 32
O = 256
NCORES = 8
BC = B // NCORES          # batches per core
TBC = T * BC              # 8 (t,b) pairs per core
HWP = H * W               # 1024
NTOT = float(T * B * H * W)  # positions per channel, global
EPS = 1e-5


def _register_lif_op():
    """Custom fused DVE op: out = (src0+src1) < s0 ? (src0+src1)*s1 : 0.
    One 1x DVE pass computing the post-reset half-carry of a LIF step.
    With s0=1, s1=0.5: out = 0.5*v*[v<1], v = u + carry_prev.
    out == 0  <=>  spike fired (v >= 1), up to the measure-zero v==0 case."""
    name = "LIF_STEP_ANT"
    for op in _dve_ops.OPS:
        if op.name == name:
            return op
    v = Src0 + Src1
    spec = _DveSpec(
        body=_dve_select(v < C0, v * C1, Zero),
        reference=lambda in0, in1, s0, s1, imm2: np.where(
            (in0.astype(np.float32) + in1) < s0,
            (in0.astype(np.float32) + in1) * s1,
            0.0,
        ).astype(np.float32),
    )
    op = _dve_ops.DveOp(name, spec, subdim=False, uops_sha={"v3": "b162af101cc4d6b9"})
    _dve_ops.OPS.append(op)
    _dve_ops.CUSTOM_DVE_SPECS[name] = spec
    _dve_ops._SUB_OPCODE_FOR_NAME[name] = (
        _dve_ops._CUSTOM_DVE_ROW_BASE + len(_dve_ops.OPS) - 1
    )
    return op


_LIF_OP = _register_lif_op()


def _build_nc():
    nc = bacc.Bacc(
        "TRN2",
        target_bir_lowering=False,
        debug=False,
        num_devices=NCORES,
    )
    xin = nc.dram_tensor("xin", [TBC, 128, 2, 2, HWP], BF, kind="ExternalInput")
    wT = nc.dram_tensor("wT", [128, 2, O], BF, kind="ExternalInput")
    gb = nc.dram_tensor("gb", [128, 6], F32, kind="ExternalInput")
    out = nc.dram_tensor("m0", [T, 2, 128, BC * HWP], F32, kind="ExternalOutput")

    xin_ap = xin.ap()
    out_ap = out.ap()

    with tile.TileContext(nc) as tc:
        with (
            tc.tile_pool(name="consts", bufs=1) as consts,
            tc.tile_pool(name="xpool", bufs=3) as xpool,
            tc.tile_pool(name="ypool", bufs=1) as ypool,
            tc.tile_pool(name="pspool", bufs=4, space="PSUM") as pspool,
            tc.tile_pool(name="scrpool", bufs=2) as scrpool,
            tc.tile_pool(name="small", bufs=1) as small,
            tc.tile_pool(name="dram", bufs=1, space="DRAM") as dram,
            tc.tile_pool(name="upool", bufs=2) as upool,
            tc.tile_pool(name="mpool", bufs=3) as mpool,
            tc.tile_pool(name="cpool", bufs=4) as cpool,
        ):
            # ---- constants ----
            w_sb = consts.tile([128, 2, O], BF, name="w_sb")
            nc.sync.dma_start(w_sb[:], wT.ap())
            gb_sb = consts.tile([128, 6], F32, name="gb_sb")
            nc.sync.dma_start(gb_sb[:], gb.ap())

            # Preload the sqrt ACT table early so the post-collective sqrt
            # doesn't pay the table-switch latency on the critical path.
            junk = small.tile([128, 1], F32, name="junk")
            nc.scalar.activation(junk[:], gb_sb[:, 0:1], ACTF.Sqrt)

            # ---- phase A: matmul + stats ----
            y_sb = [
                ypool.tile([128, TBC * HWP], F32, name=f"ysb{ot}") for ot in range(2)
            ]
            # per-(ot,tb) accumulator columns
            sums = small.tile([128, 2 * TBC], F32, name="sums")
            ssqs = small.tile([128, 2 * TBC], F32, name="ssqs")

            for tb in range(TBC):
                x_t = xpool.tile([128, 2, 2, HWP], BF, name="x_t", tag="x_t")
                nc.sync.dma_start(x_t[:], xin_ap[tb])
                for ot in range(2):
                    ps = pspool.tile([128, HWP], F32, name="ps", tag="ps")
                    for chunk in range(2):
                        nacc = 0
                        for ch in range(2):
                            for part in range(2):
                                nc.tensor.matmul(
                                    ps[:, chunk * 512 : (chunk + 1) * 512],
                                    lhsT=w_sb[:, ch, ot * 128 : (ot + 1) * 128],
                                    rhs=x_t[:, part, ch, chunk * 512 : (chunk + 1) * 512],
                                    start=(nacc == 0),
                                    stop=(nacc == 3),
                                )
                                nacc += 1
                    ysl = y_sb[ot][:, tb * HWP : (tb + 1) * HWP]
                    col = 2 * tb + ot
                    # evict PSUM -> SBUF in real units (scale_q) + free row-sum
                    nc.scalar.activation(
                        ysl,
                        ps[:],
                        ACTF.Copy,
                        bias=0.0,
                        scale=gb_sb[:, 4:5],
                        accum_out=sums[:, col : col + 1],
                    )
                    # sum of squares in one DVE pass (output discarded)
                    scr = scrpool.tile([128, HWP], F32, name="scr", tag="scr")
                    nc.vector.scalar_tensor_tensor(
                        out=scr[:],
                        in0=ysl,
                        scalar=0.0,
                        in1=ysl,
                        op0=ALU.bypass,
                        op1=ALU.mult,
                        accum_out=ssqs[:, col : col + 1],
                    )

            # ---- finalize local stats, AllReduce ----
            stats4 = small.tile([128, 4], F32, name="stats4")
            nc.vector.tensor_reduce(
                stats4[:, 0:2],
                sums[:].rearrange("p (t o) -> p o t", o=2),
                axis=mybir.AxisListType.X,
                op=ALU.add,
            )
            nc.vector.tensor_reduce(
                stats4[:, 2:4],
                ssqs[:].rearrange("p (t o) -> p o t", o=2),
                axis=mybir.AxisListType.X,
                op=ALU.add,
            )

            cc_in = dram.tile([128, 4], F32, name="cc_in")
            cc_out = dram.tile([NCORES, 128, 4], F32, name="cc_out")
            nc.gpsimd.dma_start(cc_in[:], stats4[:])
            nc.gpsimd.collective_compute(
                "AllGather",
                ALU.bypass,
                replica_groups=[list(range(NCORES))],
                ins=[cc_in.opt()],
                outs=[cc_out.opt()],
            )
            g8 = small.tile([128, NCORES, 4], F32, name="g8")
            nc.gpsimd.dma_start(g8[:], cc_out[:].rearrange("r p c -> p r c"))
            gstat = small.tile([128, 4], F32, name="gstat")
            nc.vector.tensor_reduce(
                gstat[:],
                g8[:].rearrange("p r c -> p c r"),
                axis=mybir.AxisListType.X,
                op=ALU.add,
            )

            # ---- small math: a = 0.5*inv, b = 0.5*(beta - mean*inv) ----
            aab = small.tile([128, 4], F32, name="aab")
            mean = small.tile([128, 2], F32, name="mean")
            e2 = small.tile([128, 2], F32, name="e2")
            msq = small.tile([128, 2], F32, name="msq")
            vare = small.tile([128, 2], F32, name="vare")
            sq = small.tile([128, 2], F32, name="sq")
            dd = small.tile([128, 2], F32, name="dd")
            t1 = small.tile([128, 2], F32, name="t1")
            inv = small.tile([128, 2], F32, name="inv")
            mi = small.tile([128, 2], F32, name="mi")
            bh = small.tile([128, 2], F32, name="bh")

            inv_n = 1.0 / NTOT
            nc.vector.tensor_scalar(mean[:], gstat[:, 0:2], inv_n, None, ALU.mult)
            nc.vector.tensor_scalar(e2[:], gstat[:, 2:4], inv_n, None, ALU.mult)
            nc.vector.tensor_tensor(msq[:], mean[:], mean[:], ALU.mult)
            # vare = e2 - mean^2 + eps
            nc.vector.scalar_tensor_tensor(
                out=t1[:], in0=msq[:], scalar=-1.0, in1=e2[:], op0=ALU.mult, op1=ALU.add
            )
            nc.vector.tensor_scalar(vare[:], t1[:], EPS, None, ALU.add)
            nc.scalar.activation(sq[:], vare[:], ACTF.Sqrt)
            # two Newton refinements: s = 0.5*(s + vare/s)
            rs = small.tile([128, 2], F32, name="rs")
            for _ in range(2):
                nc.vector.reciprocal(rs[:], sq[:])
                nc.vector.tensor_tensor(dd[:], vare[:], rs[:], ALU.mult)
                nc.vector.tensor_tensor(t1[:], sq[:], dd[:], ALU.add)
                nc.vector.tensor_scalar(sq[:], t1[:], 0.5, None, ALU.mult)
            # inv = gamma / sqrt(var+eps)
            nc.vector.reciprocal(rs[:], sq[:])
            nc.vector.tensor_tensor(inv[:], gb_sb[:, 0:2], rs[:], ALU.mult)
            nc.vector.tensor_scalar(aab[:, 0:2], inv[:], 0.5, None, ALU.mult)
            nc.vector.tensor_tensor(mi[:], mean[:], inv[:], ALU.mult)
            nc.vector.tensor_scalar(bh[:], gb_sb[:, 2:4], 0.5, None, ALU.mult)
            nc.vector.scalar_tensor_tensor(
                out=aab[:, 2:4], in0=mi[:], scalar=-0.5, in1=bh[:], op0=ALU.mult, op1=ALU.add
            )

            # ---- phase B: LIF over T ----
            # carry_t = 0.5*v*[v<1] with v = u_t + carry_{t-1}, one fused DVE
            # op per (t, ot). carry==0 <=> spike; host maps (carry==0)->1.0.
            zc = consts.tile([128, BC * HWP], F32, name="zc")
            nc.vector.memset(zc[:], 0.0)
            carry = [zc, zc]
            for t in range(T):
                for ot in range(2):
                    ysl = y_sb[ot][:, t * BC * HWP : (t + 1) * BC * HWP]
                    u = upool.tile([128, BC * HWP], F32, name="u", tag="u")
                    nc.scalar.activation(
                        u[:],
                        ysl,
                        ACTF.Identity,
                        bias=aab[:, 2 + ot : 3 + ot],
                        scale=aab[:, ot : ot + 1],
                    )
                    cnew = cpool.tile([128, BC * HWP], F32, name="carry", tag="carry")
                    nc.vector._custom_dve(
                        _LIF_OP, out=cnew[:], in0=u[:], in1=carry[ot][:], s0=1.0, s1=0.5
                    )
                    nc.sync.dma_start(out_ap[t, ot], cnew[:])
                    carry[ot] = cnew

    nc.compile()
    return nc


_NC_CACHE = None


def _get_nc():
    global _NC_CACHE
    if _NC_CACHE is None:
        _NC_CACHE = _build_nc()
    return _NC_CACHE


def _prep_inputs(x, w, gamma, beta):
    x = np.ascontiguousarray(np.asarray(x, dtype=np.float32))
    w = np.asarray(w, dtype=np.float32)
    gamma = np.asarray(gamma, dtype=np.float32)
    beta = np.asarray(beta, dtype=np.float32)

    # fake-quant weights exactly like the reference forward pass
    scale = (np.max(np.abs(w)) / np.float32(127.0)).astype(np.float32)
    wint = np.clip(np.rint((w / scale).astype(np.float32)), -127.0, 127.0).astype(
        np.float32
    )
    # lhsT layout: [cc(128), ch(2), O]  (w_int values are exact in bf16)
    wT_packed = np.ascontiguousarray(
        wint.T.reshape(2, 128, O).transpose(1, 0, 2)
    ).astype(BF16)

    gb_packed = np.zeros((128, 6), np.float32)
    gb_packed[:, 0] = gamma[:128]
    gb_packed[:, 1] = gamma[128:]
    gb_packed[:, 2] = beta[:128]
    gb_packed[:, 3] = beta[128:]
    gb_packed[:, 4] = scale

    xs = x.reshape(T, NCORES, BC, C, HWP)
    in_maps = []
    for c in range(NCORES):
        xf = np.ascontiguousarray(xs[:, c]).reshape(T * BC, 2, 128, HWP)
        hi = xf.astype(BF16)
        lo = (xf - hi.astype(np.float32)).astype(BF16)
        xin = np.stack([hi, lo], axis=1)  # [tb, part, ch, cc, hw]
        xin = np.ascontiguousarray(xin.transpose(0, 3, 1, 2, 4))  # [tb, cc, part, ch, hw]
        in_maps.append({"xin": xin, "wT": wT_packed, "gb": gb_packed})
    return in_maps


def _assemble(results):
    spikes = np.empty((T, B, O, H, W), np.float32)
    for c in range(NCORES):
        m = results[c]["m0"]  # [T, 2, 128, BC*HWP] f32 carry; ==0 <=> spike
        s = (m == 0).astype(np.float32)
        sm = s.reshape(T, 2, 128, BC, HWP).transpose(0, 3, 1, 2, 4)  # [t,b,ot,oc,hw]
        spikes[:, c * BC : (c + 1) * BC] = sm.reshape(T, BC, O, H, W)
    return spikes


def run(x, w, gamma, beta, trace=False, **spmd_kwargs):
    in_maps = _prep_inputs(x, w, gamma, beta)
    nc = _get_nc()
    res = bass_utils.run_bass_kernel_spmd(
        nc, in_maps, core_ids=list(range(NCORES)), trace=trace, **spmd_kwargs
    )
    return _assemble(res.results), res


def kernel(x, w, gamma, beta):
    spikes, _ = run(x, w, gamma, beta)
    return spikes


# revision 14
# speedup vs baseline: 3.2532x; 1.1272x over previous
"""Trainium2 Bass kernel for quantized 1x1-conv + BatchNorm(train) + MultiStepLIF.

Strategy (8 NeuronCores, data-parallel over batch B=16 -> 2 per core):
  y[t,b,o,hw] = sum_c x[t,b,c,hw] * w_int[o,c] * scale_q      (PE, bf16 hi/lo split)
  BN stats (mean/var per o over all t,b,hw)                   (free accum on ACT evict +
                                                               one DVE pass for sum-sq,
                                                               AllReduce of 2x256 floats)
  z = y*inv + (beta - mean*inv);  LIF: v=(v_prev+z)/2, s=[v>=1], reset
  Output per element: m0 = [v < 1] in bf16 (exact complement of spike).

Precision: x is split hi/lo into two bf16 streams (w_int is exact in bf16),
accumulated in fp32 PSUM -> ~1e-5 relative error on y, i.e. fp32-grade.
All LIF/BN elementwise work is fp32.
"""

import os
import sys

for _p in ("/opt/trn_rl_repo",):
    if _p not in sys.path:
        sys.path.insert(0, _p)

import numpy as np
import ml_dtypes

import concourse.bass as bass
import concourse.mybir as mybir
import concourse.tile as tile
from concourse import bacc
from concourse import bass_utils
from concourse import dve_ops as _dve_ops
from concourse.dve_spec import C0, C1, Spec as _DveSpec, Src0, Src1, Zero, select as _dve_select

BF16 = ml_dtypes.bfloat16
F32 = mybir.dt.float32
BF = mybir.dt.bfloat16
ALU = mybir.AluOpType
ACTF = mybir.ActivationFunctionType

T, B, C, H, W = 4, 16, 256, 32, 32
O = 256
NCORES = 8
BC = B // NCORES          # batches per core
TBC = T * BC              # 8 (t,b) pairs per core
HWP = H * W               # 1024
NTOT = float(T * B * H * W)  # positions per channel, global
EPS = 1e-5


def _register_lif_op():
    """Custom fused DVE op: out = (src0+src1) < s0 ? (src0+src1)*s1 : 0.
    One 1x DVE pass computing the post-reset half-carry of a LIF step.
    With s0=1, s1=0.5: out = 0.5*v*[v<1], v = u + carry_prev.
    out == 0  <=>  spike fired (v >= 1), up to the measure-zero v==0 case."""
    name = "LIF_STEP_ANT"
    for op in _dve_ops.OPS:
        if op.name == name:
            return op
    v = Src0 + Src1
    spec = _DveSpec(
        body=_dve_select(v < C0, v * C1, Zero),
        reference=lambda in0, in1, s0, s1, imm2: np.where(
            (in0.astype(np.float32) + in1) < s0,
            (in0.astype(np.float32) + in1) * s1,
            0.0,
        ).astype(np.float32),
    )
    op = _dve_ops.DveOp(name, spec, subdim=False, uops_sha={"v3": "b162af101cc4d6b9"})
    _dve_ops.OPS.append(op)
    _dve_ops.CUSTOM_DVE_SPECS[name] = spec
    _dve_ops._SUB_OPCODE_FOR_NAME[name] = (
        _dve_ops._CUSTOM_DVE_ROW_BASE + len(_dve_ops.OPS) - 1
    )
    return op


_LIF_OP = _register_lif_op()


def _build_nc():
    nc = bacc.Bacc(
        "TRN2",
        target_bir_lowering=False,
        debug=False,
        num_devices=NCORES,
    )
    xin = nc.dram_tensor("xin", [TBC, 128, 2, 2, HWP], BF, kind="ExternalInput")
    wT = nc.dram_tensor("wT", [128, 2, O], BF, kind="ExternalInput")
    gb = nc.dram_tensor("gb", [128, 6], F32, kind="ExternalInput")
    out = nc.dram_tensor("m0", [T, 2, 128, BC * HWP], BF, kind="ExternalOutput")

    xin_ap = xin.ap()
    out_ap = out.ap()

    with tile.TileContext(nc) as tc:
        with (
            tc.tile_pool(name="consts", bufs=1) as consts,
            tc.tile_pool(name="xpool", bufs=3) as xpool,
            tc.tile_pool(name="ypool", bufs=1) as ypool,
            tc.tile_pool(name="pspool", bufs=4, space="PSUM") as pspool,
            tc.tile_pool(name="scrpool", bufs=2) as scrpool,
            tc.tile_pool(name="small", bufs=1) as small,
            tc.tile_pool(name="dram", bufs=1, space="DRAM") as dram,
            tc.tile_pool(name="upool", bufs=2) as upool,
            tc.tile_pool(name="mpool", bufs=3) as mpool,
            tc.tile_pool(name="cpool", bufs=4) as cpool,
        ):
            # ---- constants ----
            w_sb = consts.tile([128, 2, O], BF, name="w_sb")
            nc.sync.dma_start(w_sb[:], wT.ap())
            gb_sb = consts.tile([128, 6], F32, name="gb_sb")
            nc.sync.dma_start(gb_sb[:], gb.ap())

            # Preload the sqrt ACT table early so the post-collective sqrt
            # doesn't pay the table-switch latency on the critical path.
            junk = small.tile([128, 1], F32, name="junk")
            nc.scalar.activation(junk[:], gb_sb[:, 0:1], ACTF.Sqrt)

            # ---- phase A: matmul + stats ----
            y_sb = [
                ypool.tile([128, TBC * HWP], F32, name=f"ysb{ot}") for ot in range(2)
            ]
            # per-(ot,tb) accumulator columns
            sums = small.tile([128, 2 * TBC], F32, name="sums")
            ssqs = small.tile([128, 2 * TBC], F32, name="ssqs")

            for tb in range(TBC):
                x_t = xpool.tile([128, 2, 2, HWP], BF, name="x_t", tag="x_t")
                nc.sync.dma_start(x_t[:], xin_ap[tb])
                for ot in range(2):
                    ps = pspool.tile([128, HWP], F32, name="ps", tag="ps")
                    # chunk innermost: consecutive matmuls share the stationary
                    # weight -> one LDWEIGHTS per (ch, part) instead of per MM
                    acc = 0
                    for ch in range(2):
                        for part in range(2):
                            for chunk in range(2):
                                nc.tensor.matmul(
                                    ps[:, chunk * 512 : (chunk + 1) * 512],
                                    lhsT=w_sb[:, ch, ot * 128 : (ot + 1) * 128],
                                    rhs=x_t[:, part, ch, chunk * 512 : (chunk + 1) * 512],
                                    start=(acc < 2),
                                    stop=(acc >= 6),
                                    skip_group_check=True,
                                )
                                acc += 1
                    ysl = y_sb[ot][:, tb * HWP : (tb + 1) * HWP]
                    col = 2 * tb + ot
                    # evict PSUM -> SBUF in real units (scale_q) + free row-sum
                    nc.scalar.activation(
                        ysl,
                        ps[:],
                        ACTF.Copy,
                        bias=0.0,
                        scale=gb_sb[:, 4:5],
                        accum_out=sums[:, col : col + 1],
                    )
                    # sum of squares in one DVE pass (output discarded)
                    scr = scrpool.tile([128, HWP], F32, name="scr", tag="scr")
                    nc.vector.scalar_tensor_tensor(
                        out=scr[:],
                        in0=ysl,
                        scalar=0.0,
                        in1=ysl,
                        op0=ALU.bypass,
                        op1=ALU.mult,
                        accum_out=ssqs[:, col : col + 1],
                    )

            # ---- finalize local stats, AllReduce ----
            stats4 = small.tile([128, 4], F32, name="stats4")
            nc.vector.tensor_reduce(
                stats4[:, 0:2],
                sums[:].rearrange("p (t o) -> p o t", o=2),
                axis=mybir.AxisListType.X,
                op=ALU.add,
            )
            nc.vector.tensor_reduce(
                stats4[:, 2:4],
                ssqs[:].rearrange("p (t o) -> p o t", o=2),
                axis=mybir.AxisListType.X,
                op=ALU.add,
            )

            cc_in = dram.tile([128, 4], F32, name="cc_in")
            cc_out = dram.tile([NCORES, 128, 4], F32, name="cc_out")
            nc.gpsimd.dma_start(cc_in[:], stats4[:])
            nc.gpsimd.collective_compute(
                "AllGather",
                ALU.bypass,
                replica_groups=[list(range(NCORES))],
                ins=[cc_in.opt()],
                outs=[cc_out.opt()],
            )
            g8 = small.tile([128, NCORES, 4], F32, name="g8")
            nc.gpsimd.dma_start(g8[:], cc_out[:].rearrange("r p c -> p r c"))
            gstat = small.tile([128, 4], F32, name="gstat")
            nc.vector.tensor_reduce(
                gstat[:],
                g8[:].rearrange("p r c -> p c r"),
                axis=mybir.AxisListType.X,
                op=ALU.add,
            )

            # ---- small math: a = 0.5*inv, b = 0.5*(beta - mean*inv) ----
            aab = small.tile([128, 4], F32, name="aab")
            mean = small.tile([128, 2], F32, name="mean")
            e2 = small.tile([128, 2], F32, name="e2")
            msq = small.tile([128, 2], F32, name="msq")
            vare = small.tile([128, 2], F32, name="vare")
            sq = small.tile([128, 2], F32, name="sq")
            dd = small.tile([128, 2], F32, name="dd")
            t1 = small.tile([128, 2], F32, name="t1")
            inv = small.tile([128, 2], F32, name="inv")
            mi = small.tile([128, 2], F32, name="mi")
            bh = small.tile([128, 2], F32, name="bh")

            inv_n = 1.0 / NTOT
            nc.vector.tensor_scalar(mean[:], gstat[:, 0:2], inv_n, None, ALU.mult)
            nc.vector.tensor_scalar(e2[:], gstat[:, 2:4], inv_n, None, ALU.mult)
            nc.vector.tensor_tensor(msq[:], mean[:], mean[:], ALU.mult)
            # vare = e2 - mean^2 + eps
            nc.vector.scalar_tensor_tensor(
                out=t1[:], in0=msq[:], scalar=-1.0, in1=e2[:], op0=ALU.mult, op1=ALU.add
            )
            nc.vector.tensor_scalar(vare[:], t1[:], EPS, None, ALU.add)
            nc.scalar.activation(sq[:], vare[:], ACTF.Sqrt)
            # two Newton refinements: s = 0.5*(s + vare/s)
            rs = small.tile([128, 2], F32, name="rs")
            for _ in range(2):
                nc.vector.reciprocal(rs[:], sq[:])
                nc.vector.tensor_tensor(dd[:], vare[:], rs[:], ALU.mult)
                nc.vector.tensor_tensor(t1[:], sq[:], dd[:], ALU.add)
                nc.vector.tensor_scalar(sq[:], t1[:], 0.5, None, ALU.mult)
            # inv = gamma / sqrt(var+eps)
            nc.vector.reciprocal(rs[:], sq[:])
            nc.vector.tensor_tensor(inv[:], gb_sb[:, 0:2], rs[:], ALU.mult)
            nc.vector.tensor_scalar(aab[:, 0:2], inv[:], 0.5, None, ALU.mult)
            nc.vector.tensor_tensor(mi[:], mean[:], inv[:], ALU.mult)
            nc.vector.tensor_scalar(bh[:], gb_sb[:, 2:4], 0.5, None, ALU.mult)
            nc.vector.scalar_tensor_tensor(
                out=aab[:, 2:4], in0=mi[:], scalar=-0.5, in1=bh[:], op0=ALU.mult, op1=ALU.add
            )

            # ---- phase B: LIF over T ----
            # carry_t = 0.5*v*[v<1] with v = u_t + carry_{t-1}, one fused DVE
            # op per (t, ot). carry==0 <=> spike; host maps (carry==0)->1.0.
            zc = consts.tile([128, BC * HWP], F32, name="zc")
            nc.vector.memset(zc[:], 0.0)
            carry = [zc, zc]
            for t in range(T):
                for ot in range(2):
                    ysl = y_sb[ot][:, t * BC * HWP : (t + 1) * BC * HWP]
                    u = upool.tile([128, BC * HWP], F32, name="u", tag="u")
                    nc.scalar.activation(
                        u[:],
                        ysl,
                        ACTF.Identity,
                        bias=aab[:, 2 + ot : 3 + ot],
                        scale=aab[:, ot : ot + 1],
                    )
                    cnew = cpool.tile([128, BC * HWP], F32, name="carry", tag="carry")
                    nc.vector._custom_dve(
                        _LIF_OP, out=cnew[:], in0=u[:], in1=carry[ot][:], s0=1.0, s1=0.5
                    )
                    # compact bf16 spike-indicator for the output DMA:
                    # m = [carry != 0] in {0,1}; m==0 <=> spike
                    m0 = mpool.tile([128, BC * HWP], BF, name="m0", tag="m0")
                    nc.vector.tensor_scalar(m0[:], cnew[:], 0.0, None, ALU.not_equal)
                    nc.sync.dma_start(out_ap[t, ot], m0[:])
                    carry[ot] = cnew

    nc.compile()
    return nc


_NC_CACHE = None


def _get_nc():
    global _NC_CACHE
    if _NC_CACHE is None:
        _NC_CACHE = _build_nc()
    return _NC_CACHE


def _prep_inputs(x, w, gamma, beta):
    x = np.ascontiguousarray(np.asarray(x, dtype=np.float32))
    w = np.asarray(w, dtype=np.float32)
    gamma = np.asarray(gamma, dtype=np.float32)
    beta = np.asarray(beta, dtype=np.float32)

    # fake-quant weights exactly like the reference forward pass
    scale = (np.max(np.abs(w)) / np.float32(127.0)).astype(np.float32)
    wint = np.clip(np.rint((w / scale).astype(np.float32)), -127.0, 127.0).astype(
        np.float32
    )
    # lhsT layout: [cc(128), ch(2), O]  (w_int values are exact in bf16)
    wT_packed = np.ascontiguousarray(
        wint.T.reshape(2, 128, O).transpose(1, 0, 2)
    ).astype(BF16)

    gb_packed = np.zeros((128, 6), np.float32)
    gb_packed[:, 0] = gamma[:128]
    gb_packed[:, 1] = gamma[128:]
    gb_packed[:, 2] = beta[:128]
    gb_packed[:, 3] = beta[128:]
    gb_packed[:, 4] = scale

    xs = x.reshape(T, NCORES, BC, C, HWP)
    in_maps = []
    for c in range(NCORES):
        xf = np.ascontiguousarray(xs[:, c]).reshape(T * BC, 2, 128, HWP)
        hi = xf.astype(BF16)
        lo = (xf - hi.astype(np.float32)).astype(BF16)
        xin = np.stack([hi, lo], axis=1)  # [tb, part, ch, cc, hw]
        xin = np.ascontiguousarray(xin.transpose(0, 3, 1, 2, 4))  # [tb, cc, part, ch, hw]
        in_maps.append({"xin": xin, "wT": wT_packed, "gb": gb_packed})
    return in_maps


def _assemble(results):
    spikes = np.empty((T, B, O, H, W), np.float32)
    for c in range(NCORES):
        m = results[c]["m0"]  # [T, 2, 128, BC*HWP] f32 carry; ==0 <=> spike
        s = (m == 0).astype(np.float32)
        sm = s.reshape(T, 2, 128, BC, HWP).transpose(0, 3, 1, 2, 4)  # [t,b,ot,oc,hw]
        spikes[:, c * BC : (c + 1) * BC] = sm.reshape(T, BC, O, H, W)
    return spikes


def run(x, w, gamma, beta, trace=False, **spmd_kwargs):
    in_maps = _prep_inputs(x, w, gamma, beta)
    nc = _get_nc()
    res = bass_utils.run_bass_kernel_spmd(
        nc, in_maps, core_ids=list(range(NCORES)), trace=trace, **spmd_kwargs
    )
    return _assemble(res.results), res


def kernel(x, w, gamma, beta):
    spikes, _ = run(x, w, gamma, beta)
    return spikes
